# revision 1
# baseline (speedup 1.0000x reference)
"""Polynomial features (degree 2) + linear layer, distributed over 8 TRN2 cores.

reference: A = [x, {x_i*x_j for i<=j}] (8384 coeffs); out = A @ W.T + b.

Device algorithm (per core, batch shard 4096, feature-on-partition layout):
  - pairs are enumerated by circular distance class s in 0..64:
      class s, lane p  ->  unordered pair {p, (p+s) % 128}
    (each unordered pair appears exactly once; s=64 lanes >=64 are dups
    with zeroed weights)
  - host ships 16 rotated copies of x^T (rot d: row p = feature (p+d)%128)
    for d in D = {0..8, 16, 24, 32, 40, 48, 56, 64}; every class s is one
    bf16 DVE tensor_mul of two rotations with b - a = s (the hardware only
    allows 32-aligned partition bases, so all ops are full 128-partition,
    base 0 - the rotations do the shifting)
  - 66 matmuls (1 linear chunk + 65 class chunks, K=128 each) accumulate
    into PSUM [64 outs, 512 batch]; W is permuted host-side to match;
    bias is added in the PSUM->SBUF copy (DVE tensor_scalar_add)
  - TPB instructions have a single sync-wait slot, but Tile emits multiple
    waits on slot-recycling instructions; _split_multiwaits() post-processes
    the BIR, hoisting extra waits onto injected same-engine NOPs
"""

import numpy as np
import ml_dtypes

INPUT_DIM = 128
OUTPUT_DIM = 64
BATCH = 32768
N_CORES = 8
B_CORE = BATCH // N_CORES  # 4096
TILE_B = 512
N_TILES = B_CORE // TILE_B  # 8

ROT_SET = [0, 1, 2, 3, 4, 5, 6, 7, 8, 16, 24, 32, 40, 48, 56, 64]
N_ROT = len(ROT_SET)
ROT_IDX = {d: i for i, d in enumerate(ROT_SET)}

import os

GPS_OP_IDS = tuple(
    int(v) for v in os.environ.get("K_GPS_OPS", "").split(",") if v != ""
)


def _class_ops():
    """(a, b) rotation pair per distance class s=0..64 with b - a = s."""
    ops = []
    for s in range(65):
        if s <= 8:
            a, b = 0, s
        else:
            k = (s - 1) // 8  # 1..7
            anchor = 8 * k + 8
            a, b = anchor - s, anchor
        assert a in ROT_SET and b in ROT_SET and b - a == s, (s, a, b)
        ops.append((a, b))
    return ops


CLASS_OPS = _class_ops()


def _build_device_weights(W, b):
    """Permute W [64, 8384] into the device K-block layout.

    Returns w_packed [128, 66*64]: block j (j=0 linear, j=1+s class s)
    lives at free columns [j*64, (j+1)*64), partition p = K row p.
    Class s row p -> pair {p, (p+s)%128}; s=64 rows p>=64 are zeroed dups.
    """
    W = np.asarray(W, np.float32)
    n = INPUT_DIM
    pair_off = {}
    c = 0
    for i in range(n):
        for j in range(i, n):
            pair_off[(i, j)] = c
            c += 1
    assert c == 8256

    Wd = np.zeros((66, 128, OUTPUT_DIM), np.float32)
    Wd[0] = W[:, 0:128].T  # linear block
    seen = set()
    for s in range(65):
        a, _bb = CLASS_OPS[s]
        for p in range(128):
            u = (p + a) % 128
            v = (p + a + s) % 128
            i, j = (u, v) if u <= v else (v, u)
            if (i, j) in seen:
                continue  # duplicate lane (s=64 second half)
            seen.add((i, j))
            Wd[1 + s, p] = W[:, 128 + pair_off[(i, j)]]
    assert len(seen) == 8256, len(seen)
    w_packed = np.ascontiguousarray(
        Wd.transpose(1, 0, 2).reshape(128, 66 * OUTPUT_DIM)
    ).astype(ml_dtypes.bfloat16)
    return w_packed, np.asarray(b, np.float32)


def _split_multiwaits(nc, mybir):
    """TPB instructions have one sync-wait slot; hoist extras onto NOPs."""
    import bass_rust

    n_split = 0
    for fn in nc.m.functions:
        for bb in fn.blocks:
            out = []
            changed = False
            for inst in bb.instructions:
                si = getattr(inst, "sync_info", None)
                if si is not None and si.on_wait and len(si.on_wait) > 1:
                    for w in si.on_wait[:-1]:
                        n_split += 1
                        nop = bass_rust.InstNoOp(
                            name=f"I-mw{n_split}",
                            engine=inst.engine,
                            ins=[],
                            outs=[],
                            sync_info=mybir.SyncInfo(on_wait=[w], on_update=[]),
                            bass_nofuse=True,
                        )
                        out.append(nop)
                    inst.sync_info = mybir.SyncInfo(
                        on_wait=[si.on_wait[-1]], on_update=si.on_update
                    )
                    changed = True
                out.append(inst)
            if changed:
                bb.instructions = out
    return n_split


def build(x, W, b):
    """Build the Bass graph and per-core input maps. Returns (nc, in_maps)."""
    import concourse.bass as bass
    import concourse.mybir as mybir
    from concourse import tile

    bf16 = mybir.dt.bfloat16
    f32 = mybir.dt.float32

    # ---- host preprocessing ----
    xT = np.ascontiguousarray(np.asarray(x, np.float32).T).astype(
        ml_dtypes.bfloat16
    )  # [128, 32768]
    # xall[p, i, n] = feature (p + ROT_SET[i]) % 128 of sample n
    xall = np.stack([np.roll(xT, -d, axis=0) for d in ROT_SET], axis=1)
    w_packed, bias = _build_device_weights(W, b)

    # ---- device graph ----
    nc = bass.Bass()
    x_in = nc.declare_dram_parameter(
        "xall", [N_TILES, 128, N_ROT, TILE_B], bf16, isOutput=False
    )
    w_in = nc.declare_dram_parameter("Wd", [128, 66 * 64], bf16, isOutput=False)
    b_in = nc.declare_dram_parameter("bias", [OUTPUT_DIM, 1], f32, isOutput=False)
    out_ext = nc.declare_dram_parameter(
        "outT", [OUTPUT_DIM, B_CORE], f32, isOutput=True
    )

    # multi-class ops: one per anchor family, constant-stride rotation APs:
    # op 0 = classes 0..8 (rot0 x rot 0..8), ops 1..7 = classes 8k+1..8k+8
    MC_OPS = [list(range(0, 9))] + [
        list(range(8 * k + 1, 8 * k + 9)) for k in range(1, 8)
    ]
    GPS_OPS = set(GPS_OP_IDS)  # op indices computed on GpSimd

    def rot_group_ap(xrt, classes):
        """[128, len(classes), TILE_B] APs (in0, in1)."""
        m = len(classes)
        us = [ROT_IDX[CLASS_OPS[s][0]] for s in classes]
        vs = [ROT_IDX[CLASS_OPS[s][1]] for s in classes]

        def mk(idx):
            if all(i == idx[0] for i in idx):
                return xrt[:, idx[0] : idx[0] + 1, :].to_broadcast(
                    [128, m, TILE_B]
                )
            d = idx[1] - idx[0]
            assert all(idx[j + 1] - idx[j] == d for j in range(m - 1)), idx
            return xrt[:, idx[0] :: d, :][:, 0:m, :]

        return mk(us), mk(vs)

    with tile.TileContext(nc) as tc:
        with (
            tc.tile_pool(name="consts", bufs=1) as consts,
            tc.tile_pool(name="xc", bufs=3) as xcp,
            tc.tile_pool(name="prod", bufs=4) as prodp,
            tc.tile_pool(name="prodg", bufs=5) as prodgp,
            tc.tile_pool(name="outp", bufs=3) as outp,
            tc.tile_pool(name="psum", bufs=2, space="PSUM") as psump,
        ):
            w_sb = consts.tile([128, 66 * 64], bf16)
            nc.sync.dma_start(w_sb[:], w_in[:])
            b_sb = consts.tile([OUTPUT_DIM, 1], f32)
            nc.sync.dma_start(b_sb[:], b_in[:])

            xc_tiles = [None] * (N_TILES + 2)

            def load_xc(t):
                if t >= N_TILES:
                    return
                xt = xcp.tile([128, N_ROT, TILE_B], bf16, tag="xc", name="xc_t")
                nc.sync.dma_start(xt[:], x_in[t][:])
                xc_tiles[t] = xt

            load_xc(0)
            load_xc(1)
            for t in range(N_TILES):
                load_xc(t + 2)
                xrt = xc_tiles[t]

                # acc halves: even classes + linear -> partitions 0:64
                # (array cols 0-63), odd classes -> partitions 64:128
                acc = psump.tile([128, TILE_B], f32, name="acc")
                nc.tensor.matmul(
                    acc[0:64, :],
                    w_sb[:, 0:64],
                    xrt[:, 0, :],
                    start=True,
                    stop=False,
                    tile_position=(0, 0),
                )
                first_odd = True
                for k, classes in enumerate(MC_OPS):
                    m = len(classes)
                    pool_k = prodgp if k in GPS_OPS else prodp
                    tag = ("prodg" if k in GPS_OPS else "prod") + str(m)
                    p_t = pool_k.tile(
                        [128, m, TILE_B], bf16, tag=tag, name="p_t"
                    )
                    in0, in1 = rot_group_ap(xrt, classes)
                    eng = nc.gpsimd if k in GPS_OPS else nc.vector
                    eng.tensor_mul(p_t[:], in0, in1)
                    views = [
                        (s, p_t[:, j, :]) for j, s in enumerate(classes)
                    ]
                    for s, rhs in views:
                        half = s % 2
                        blk = 1 + s
                        is_last_even = s == 64
                        is_last_odd = s == 63
                        nc.tensor.matmul(
                            acc[64 * half : 64 * half + 64, :],
                            w_sb[:, blk * 64 : (blk + 1) * 64],
                            rhs,
                            start=(half == 1 and first_odd),
                            stop=(is_last_even or is_last_odd),
                            tile_position=(0, 64 * half),
                        )
                        if half == 1:
                            first_odd = False

                # ACT evacuates both PSUM halves; accumulating DMA adds the
                # odd half into DRAM (keeps DVE free for products)
                o_t = outp.tile([OUTPUT_DIM, TILE_B], f32, tag="o", name="o_t")
                o2_t = outp.tile([OUTPUT_DIM, TILE_B], f32, tag="o2", name="o2_t")
                nc.scalar.activation(
                    o_t[:],
                    acc[0:64, :],
                    mybir.ActivationFunctionType.Identity,
                    bias=b_sb[:, 0:1],
                )
                nc.scalar.copy(o2_t[:], acc[64:128, :])
                bs = slice(t * TILE_B, (t + 1) * TILE_B)
                nc.sync.dma_start(out_ext[:, bs], o_t[:])
                nc.gpsimd.dma_start(
                    out_ext[:, bs], o2_t[:], accum_op=mybir.AluOpType.add
                )

    _split_multiwaits(nc, mybir)

    # ---- per-core input maps ----
    in_maps = []
    for c in range(N_CORES):
        cs = xall[:, :, c * B_CORE : (c + 1) * B_CORE]  # [128, 16, 4096]
        xtiles = np.ascontiguousarray(
            cs.reshape(128, N_ROT, N_TILES, TILE_B).transpose(2, 0, 1, 3)
        )  # [N_TILES, 128, 16, TILE_B]
        in_maps.append(
            {
                "xall": xtiles,
                "Wd": w_packed,
                "bias": bias.reshape(OUTPUT_DIM, 1),
            }
        )
    return nc, in_maps


def kernel(x, W, b, indices_0, indices_1):
    from concourse.bass_utils import run_bass_kernel_spmd

    nc, in_maps = build(x, W, b)
    res = run_bass_kernel_spmd(nc, in_maps, list(range(N_CORES))).results
    out = np.concatenate([np.asarray(r["outT"], np.float32).T for r in res], axis=0)
    return out



# revision 5
# speedup vs baseline: 1.0829x; 1.0829x over previous
"""Polynomial features (degree 2) + linear layer, distributed over 8 TRN2 cores.

reference: A = [x, {x_i*x_j for i<=j}] (8384 coeffs); out = A @ W.T + b.

Device algorithm (per core, batch shard 4096, feature-on-partition layout):
  - pairs are enumerated by circular distance class s in 0..64:
      class s, lane p  ->  unordered pair {p, (p+s) % 128}
    (each unordered pair appears exactly once; s=64 lanes >=64 are dups
    with zeroed weights)
  - class products are computed three ways, balancing DVE / GpSimd /
    PE+ACT load:
      * DVE classes: bf16 tensor_mul of two rotated copies of x^T
        (rot d: row p = feature (p+d)%128), shipped from host
      * GPS classes: same multiply, on GpSimd
      * POLAR classes (anchor families 56, 64 by default): polarization
        x_a*x_b = ((x_a+x_b)^2 - x_a^2 - x_b^2)/2. The sum x_a+x_b is a
        PE matmul with a 0/1 permutation-sum stationary matrix against
        un-rotated x; ACT evacuates PSUM with Square -> bf16 q_s; the
        contraction uses W_s/2; the -x_a^2-x_b^2 corrections fold into
        the class-0 (squares) weight block applied to SQ = x^2 (ACT
        Square of rot-0). Polar anchors' rotations need not be shipped.
  - 66 contraction matmuls (linear + SQ + 64 classes, K=128 each)
    accumulate into PSUM halves (even classes + linear + SQ ->
    partitions 0:64 / array cols 0:64, odd -> 64:128); a final identity
    matmul folds the odd half (ACT-copied to SBUF bf16) into the even
    accumulation; ACT adds bias during the single PSUM->SBUF copy; one
    plain DMA per tile writes out
  - TPB instructions have a single sync-wait slot, but Tile emits multiple
    waits on slot-recycling instructions; _split_multiwaits() post-processes
    the BIR, hoisting extra waits onto injected same-engine NOPs
"""

import os

import numpy as np
import ml_dtypes

INPUT_DIM = 128
OUTPUT_DIM = 64
BATCH = 32768
N_CORES = 8
B_CORE = BATCH // N_CORES  # 4096
TILE_B = 512
N_TILES = B_CORE // TILE_B  # 8

ALL_ANCHORS = (16, 24, 32, 40, 48, 56, 64)

# Tuning knobs (defaults hardcoded for the graded kernel; env overridable
# for experiments).
POLAR_ANCHORS = tuple(
    int(v)
    for v in os.environ.get("K_POLAR_ANCHORS", "56,64").split(",")
    if v != ""
)
GPS_N = int(os.environ.get("K_GPS_N", "4"))  # classes off the last DVE family

POLAR_CLASSES = tuple(s for a in POLAR_ANCHORS for s in range(a - 7, a + 1))
ROT_SET = list(range(9)) + [a for a in ALL_ANCHORS if a not in POLAR_ANCHORS]
N_ROT = len(ROT_SET)
ROT_IDX = {d: i for i, d in enumerate(ROT_SET)}


def _class_ops():
    """(a, b) rotation pair per distance class s=0..64 with b - a = s."""
    ops = []
    for s in range(65):
        if s <= 8:
            a, b = 0, s
        else:
            k = (s - 1) // 8  # 1..7
            anchor = 8 * k + 8
            a, b = anchor - s, anchor
        ops.append((a, b))
    return ops


CLASS_OPS = _class_ops()


def _mul_groups():
    """DVE and GPS multi-class product ops: (classes, on_gps) in issue order.

    Constant-stride rotation APs: classes 1..8 use rot0 x rot 1..8; each
    non-polar anchor family 8k+1..8k+8 uses rots [7..0] x anchor. The last
    family donates its final GPS_N classes to GpSimd.
    """
    fams = [list(range(1, 9))]
    for a in ALL_ANCHORS:
        if a not in POLAR_ANCHORS:
            fams.append(list(range(a - 7, a + 1)))
    groups = []
    for i, fam in enumerate(fams):
        last = i == len(fams) - 1
        if last and 0 < GPS_N < len(fam):
            groups.append((fam[:-GPS_N], False))
            groups.append((fam[-GPS_N:], True))
        else:
            groups.append((fam, False))
    return groups


MUL_GROUPS = _mul_groups()


def _build_device_weights(W, b):
    """Permute W [64, 8384] into the device K-block layout.

    Returns (w_packed [128, 66*64] bf16, s_packed [128, n_pol*128] bf16,
    bias f32). Block j=0 linear, j=1 SQ (class 0 + polar corrections),
    j=1+s class s (scaled 1/2 for polar classes). Class s row p -> pair
    {(p+a)%128, (p+a+s)%128}; duplicate lanes (s=64 second half) zeroed.
    """
    W = np.asarray(W, np.float32)
    n = INPUT_DIM
    pair_off = {}
    c = 0
    for i in range(n):
        for j in range(i, n):
            pair_off[(i, j)] = c
            c += 1
    assert c == 8256

    Wl = np.zeros((65, 128, OUTPUT_DIM), np.float32)
    seen = set()
    for s in range(65):
        a, _bb = CLASS_OPS[s]
        for p in range(128):
            u = (p + a) % 128
            v = (p + a + s) % 128
            i, j = (u, v) if u <= v else (v, u)
            if (i, j) in seen:
                continue  # duplicate lane (s=64 second half)
            seen.add((i, j))
            Wl[s, p] = W[:, 128 + pair_off[(i, j)]]
    assert len(seen) == 8256, len(seen)

    # polarization corrections: -1/2 sum_s (W_s scattered to x_a^2, x_b^2 lanes)
    C = np.zeros((128, OUTPUT_DIM), np.float32)
    for s in POLAR_CLASSES:
        a, bb = CLASS_OPS[s]
        for p in range(128):
            C[(p + a) % 128] += Wl[s, p]
            C[(p + bb) % 128] += Wl[s, p]
    C *= -0.5

    blocks = np.zeros((66, 128, OUTPUT_DIM), np.float32)
    blocks[0] = W[:, 0:128].T  # linear
    blocks[1] = Wl[0] + C  # SQ block
    for s in range(1, 65):
        blocks[1 + s] = Wl[s] * (0.5 if s in POLAR_CLASSES else 1.0)
    w_packed = np.ascontiguousarray(
        blocks.transpose(1, 0, 2).reshape(128, 66 * OUTPUT_DIM)
    ).astype(ml_dtypes.bfloat16)

    # 0/1 permutation-sum stationary matrices for polar classes:
    # out[p, n] = x[(p+a)%128, n] + x[(p+b)%128, n]
    n_pol = len(POLAR_CLASSES)
    S = np.zeros((max(n_pol, 1), 128, 128), np.float32)
    for i, s in enumerate(POLAR_CLASSES):
        a, bb = CLASS_OPS[s]
        for p in range(128):
            S[i, (p + a) % 128, p] += 1.0
            S[i, (p + bb) % 128, p] += 1.0
    s_packed = np.ascontiguousarray(
        S.transpose(1, 0, 2).reshape(128, max(n_pol, 1) * 128)
    ).astype(ml_dtypes.bfloat16)

    return w_packed, s_packed, np.asarray(b, np.float32)


def _split_multiwaits(nc, mybir):
    """TPB instructions have one sync-wait slot; hoist extras onto NOPs."""
    import bass_rust

    n_split = 0
    for fn in nc.m.functions:
        for bb in fn.blocks:
            out = []
            changed = False
            for inst in bb.instructions:
                si = getattr(inst, "sync_info", None)
                if si is not None and si.on_wait and len(si.on_wait) > 1:
                    for w in si.on_wait[:-1]:
                        n_split += 1
                        nop = bass_rust.InstNoOp(
                            name=f"I-mw{n_split}",
                            engine=inst.engine,
                            ins=[],
                            outs=[],
                            sync_info=mybir.SyncInfo(on_wait=[w], on_update=[]),
                            bass_nofuse=True,
                        )
                        out.append(nop)
                    inst.sync_info = mybir.SyncInfo(
                        on_wait=[si.on_wait[-1]], on_update=si.on_update
                    )
                    changed = True
                out.append(inst)
            if changed:
                bb.instructions = out
    return n_split


def build(x, W, b):
    """Build the Bass graph and per-core input maps. Returns (nc, in_maps)."""
    import concourse.bass as bass
    import concourse.mybir as mybir
    from concourse import tile

    bf16 = mybir.dt.bfloat16
    f32 = mybir.dt.float32

    n_pol = len(POLAR_CLASSES)
    assert n_pol % 2 == 0
    n_chunk = n_pol // 2

    # ---- host preprocessing ----
    xT = np.ascontiguousarray(np.asarray(x, np.float32).T).astype(
        ml_dtypes.bfloat16
    )  # [128, 32768]
    # xall[p, i, n] = feature (p + ROT_SET[i]) % 128 of sample n
    xall = np.stack([np.roll(xT, -d, axis=0) for d in ROT_SET], axis=1)
    w_packed, s_packed, bias = _build_device_weights(W, b)

    # ---- device graph ----
    nc = bass.Bass()
    x_in = nc.declare_dram_parameter(
        "xall", [N_TILES, 128, N_ROT, TILE_B], bf16, isOutput=False
    )
    w_in = nc.declare_dram_parameter("Wd", [128, 66 * 64], bf16, isOutput=False)
    s_in = nc.declare_dram_parameter(
        "Ssum", [128, max(n_pol, 1) * 128], bf16, isOutput=False
    )
    i_in = nc.declare_dram_parameter(
        "I64", [OUTPUT_DIM, OUTPUT_DIM], bf16, isOutput=False
    )
    b_in = nc.declare_dram_parameter("bias", [OUTPUT_DIM, 1], f32, isOutput=False)
    out_ext = nc.declare_dram_parameter(
        "outT", [OUTPUT_DIM, B_CORE], f32, isOutput=True
    )

    def rot_group_ap(xrt, classes):
        """[128, len(classes), TILE_B] APs (in0, in1)."""
        m = len(classes)
        us = [ROT_IDX[CLASS_OPS[s][0]] for s in classes]
        vs = [ROT_IDX[CLASS_OPS[s][1]] for s in classes]

        def mk(idx):
            if all(i == idx[0] for i in idx):
                return xrt[:, idx[0] : idx[0] + 1, :].to_broadcast(
                    [128, m, TILE_B]
                )
            d = idx[1] - idx[0]
            assert all(idx[j + 1] - idx[j] == d for j in range(m - 1)), idx
            return xrt[:, idx[0] :: d, :][:, 0:m, :]

        return mk(us), mk(vs)

    with tile.TileContext(nc) as tc:
        with (
            tc.tile_pool(name="consts", bufs=1) as consts,
            tc.tile_pool(name="xc", bufs=3) as xcp,
            tc.tile_pool(name="prod", bufs=4) as prodp,
            tc.tile_pool(name="prodg", bufs=4) as prodgp,
            tc.tile_pool(name="sq", bufs=3) as sqp,
            tc.tile_pool(name="q", bufs=3) as qp,
            tc.tile_pool(name="outp", bufs=3) as outp,
            tc.tile_pool(name="psum", bufs=2, space="PSUM") as psump,
            tc.tile_pool(name="psum_s", bufs=3, space="PSUM") as psump_s,
        ):
            xc_tiles = [None] * (N_TILES + 2)

            def load_xc(t, first=False):
                if t >= N_TILES:
                    return
                xt = xcp.tile([128, N_ROT, TILE_B], bf16, tag="xc", name="xc_t")
                if first:
                    # split so the first product op starts after rots 0..8
                    nc.sync.dma_start(xt[:, 0:9, :], x_in[t][:, 0:9, :])
                    nc.sync.dma_start(xt[:, 9:N_ROT, :], x_in[t][:, 9:N_ROT, :])
                else:
                    nc.sync.dma_start(xt[:], x_in[t][:])
                xc_tiles[t] = xt

            load_xc(0, first=True)
            s_sb = consts.tile([128, max(n_pol, 1) * 128], bf16)
            nc.sync.dma_start(s_sb[:], s_in[:])
            w_sb = consts.tile([128, 66 * 64], bf16)
            nc.sync.dma_start(w_sb[:], w_in[:])
            b_sb = consts.tile([OUTPUT_DIM, 1], f32)
            nc.sync.dma_start(b_sb[:], b_in[:])
            i64_sb = consts.tile([OUTPUT_DIM, OUTPUT_DIM], bf16)
            nc.sync.dma_start(i64_sb[:], i_in[:])
            load_xc(1)

            for t in range(N_TILES):
                load_xc(t + 2)
                xrt = xc_tiles[t]

                # SQ = x^2 (rot 0) on ACT
                sq_t = sqp.tile([128, TILE_B], bf16, tag="sq", name="sq_t")
                nc.scalar.activation(
                    sq_t[:],
                    xrt[:, 0, :],
                    mybir.ActivationFunctionType.Square,
                )

                # polar sums on PE -> PSUM; ACT squares into bf16 q chunks
                q_tiles = []

                def sum_chunk(cidx):
                    ps = psump_s.tile(
                        [128, 2, TILE_B], f32, tag="ps", name="ps_t"
                    )
                    for j in range(2):
                        i = 2 * cidx + j
                        nc.tensor.matmul(
                            ps[:, j, :],
                            s_sb[:, i * 128 : (i + 1) * 128],
                            xrt[:, 0, :],
                            start=True,
                            stop=True,
                        )
                    q_t = qp.tile([128, 2, TILE_B], bf16, tag="q", name="q_t")
                    nc.scalar.activation(
                        q_t[:],
                        ps[:],
                        mybir.ActivationFunctionType.Square,
                    )
                    q_tiles.append(q_t)

                next_chunk = 0

                def issue_chunks(n):
                    nonlocal next_chunk
                    for _ in range(n):
                        if next_chunk < n_chunk:
                            sum_chunk(next_chunk)
                            next_chunk += 1

                issue_chunks(3)

                # direct product groups on DVE / GPS (issue all muls now;
                # their engines run ahead independently)
                group_tiles = []
                for classes, on_gps in MUL_GROUPS:
                    m = len(classes)
                    pool_k = prodgp if on_gps else prodp
                    tag = ("prodg" if on_gps else "prod") + str(m)
                    p_t = pool_k.tile(
                        [128, m, TILE_B], bf16, tag=tag, name="p_t"
                    )
                    in0, in1 = rot_group_ap(xrt, classes)
                    eng = nc.gpsimd if on_gps else nc.vector
                    eng.tensor_mul(p_t[:], in0, in1)
                    group_tiles.append((classes, p_t))

                # contraction accumulation into PSUM halves
                acc = psump.tile([128, TILE_B], f32, name="acc")
                nc.tensor.matmul(
                    acc[0:64, :],
                    w_sb[:, 0:64],
                    xrt[:, 0, :],
                    start=True,
                    stop=False,
                    tile_position=(0, 0),
                )
                nc.tensor.matmul(
                    acc[0:64, :],
                    w_sb[:, 64:128],
                    sq_t[:],
                    start=False,
                    stop=False,
                    tile_position=(0, 0),
                )

                # ordered contraction operands; polar q chunks read lazily
                # (their tiles exist only after issue_chunks ran)
                contraction = []
                for classes, p_t in group_tiles:
                    for j, s in enumerate(classes):
                        contraction.append((s, (0, p_t, j)))
                for cidx in range(n_chunk):
                    for j in range(2):
                        contraction.append(
                            (POLAR_CLASSES[2 * cidx + j], (1, cidx, j))
                        )
                last_odd_i = max(
                    (i for i, (s, _) in enumerate(contraction) if s % 2 == 1),
                    default=None,
                )

                first_odd = True
                for i, (s, ref) in enumerate(contraction):
                    if ref[0] == 0:
                        _, p_t, j = ref
                        rhs = p_t[:, j, :]
                    else:
                        _, cidx, j = ref
                        issue_chunks(cidx + 1 - next_chunk)
                        rhs = q_tiles[cidx][:, j, :]
                    half = s % 2
                    blk = 1 + s
                    nc.tensor.matmul(
                        acc[64 * half : 64 * half + 64, :],
                        w_sb[:, blk * 64 : (blk + 1) * 64],
                        rhs,
                        start=(half == 1 and first_odd),
                        stop=(i == last_odd_i),
                        tile_position=(0, 64 * half),
                    )
                    if half == 1:
                        first_odd = False
                    # interleave remaining sum chunks among the direct groups
                    if ref[0] == 0 and (i + 1) % 8 == 0:
                        issue_chunks(1)

                # fold odd half into even accumulation via identity matmul
                o2_t = outp.tile(
                    [OUTPUT_DIM, TILE_B], bf16, tag="o2", name="o2_t"
                )
                nc.scalar.copy(o2_t[:], acc[64:128, :])
                nc.tensor.matmul(
                    acc[0:64, :],
                    i64_sb[:],
                    o2_t[:],
                    start=False,
                    stop=True,
                    tile_position=(0, 0),
                )
                o_t = outp.tile([OUTPUT_DIM, TILE_B], f32, tag="o", name="o_t")
                nc.scalar.activation(
                    o_t[:],
                    acc[0:64, :],
                    mybir.ActivationFunctionType.Identity,
                    bias=b_sb[:, 0:1],
                )
                bs = slice(t * TILE_B, (t + 1) * TILE_B)
                nc.sync.dma_start(out_ext[:, bs], o_t[:])

    _split_multiwaits(nc, mybir)

    # ---- per-core input maps ----
    i64 = np.eye(OUTPUT_DIM, dtype=np.float32).astype(ml_dtypes.bfloat16)
    in_maps = []
    for c in range(N_CORES):
        cs = xall[:, :, c * B_CORE : (c + 1) * B_CORE]  # [128, N_ROT, 4096]
        xtiles = np.ascontiguousarray(
            cs.reshape(128, N_ROT, N_TILES, TILE_B).transpose(2, 0, 1, 3)
        )  # [N_TILES, 128, N_ROT, TILE_B]
        in_maps.append(
            {
                "xall": xtiles,
                "Wd": w_packed,
                "Ssum": s_packed,
                "I64": i64,
                "bias": bias.reshape(OUTPUT_DIM, 1),
            }
        )
    return nc, in_maps


def kernel(x, W, b, indices_0, indices_1):
    from concourse.bass_utils import run_bass_kernel_spmd

    nc, in_maps = build(x, W, b)
    res = run_bass_kernel_spmd(nc, in_maps, list(range(N_CORES))).results
    out = np.concatenate([np.asarray(r["outT"], np.float32).T for r in res], axis=0)
    return out
